# revision 2
# baseline (speedup 1.0000x reference)
"""Multi-head attention (B=4, L=2048, C=1024, H=16, HD=64) on 8 NeuronCores.

Sharding: tensor-parallel over heads - 2 heads per core. Each core computes
its heads' QKV projection, attention, and a partial output projection over
its 128 ctx channels; the host sums the 8 partial outputs (b_out added on
host). All HW-side matmul operands are bf16; PSUM accumulation stays fp32.

This version processes BOTH heads concurrently through PE array tiling
(validated on HW in a standalone minitest):
  - scores: 2x row tiling (64x128). Head h's k-strip [64ch x 128ktok] is the
    stationary at tile_position (64h, 0); its q [64ch, 512qtok] streams from
    SBUF partitions 64h..64h+64. Both heads' matmuls run concurrently ->
    scores at ~full PE efficiency (vs 50% with the 128-frame zero-padding).
  - attnV: 4x column tiling (128x32). v strips [128ktok x 32ch] at
    tile_position (0, 32j) accumulate ctx for both heads into ONE psum bank
    X = [h0 ch0:64 | h1 ch0:64]. The softmax rowsum rides as M=1 "ones"
    strips into bank Y (rows 0/32 = h0 even/odd ktiles, 64/96 = h1),
    alternating positions per ktile so每 position carries 1.5 streams.
  - Z path: DVE copy Y->SBUF (lane-aligned), DMA-gather rows {32,64,96} to
    partition 0 (HW partition_broadcast requires base-0 APs - measured),
    add, gpsimd-broadcast per head, DMA-replicate h1's block to partitions
    64:128, one reciprocal_approx_fast [128,512] (DVE custom op, ~18-bit),
    one full-width normalize mul. ACT does exp ONLY.

Pipeline: one flat stream of 256 (group, ktile) steps; scores run 2 steps
ahead of their exp, attnV trails its exp, and qkv projection / v transposes
/ output projection are emitted as filler slices on a fixed schedule so the
in-order PE queue always has ready work while ACT (the pacing engine,
~1.01us per [128,1024] exp) streams. PSUM: s 2x[128,1024] (4 banks) +
X 2x[128,512] + Y + G(filler) = 8 banks exactly.
"""

import numpy as np
import ml_dtypes

import concourse.bass as bass
import concourse.mybir as mybir
import concourse.tile as tile
from concourse import bacc
from concourse.bass_utils import run_bass_kernel_spmd

B, L, C, H, HD = 4, 2048, 1024, 16, 64
NCORES = 8
HPC = H // NCORES  # heads per core = 2
F32 = mybir.dt.float32
BF16 = mybir.dt.bfloat16
EXP = mybir.ActivationFunctionType.Exp

LCHUNK = 512          # token chunk for qkv projection drains
NLC = L // LCHUNK     # 4
NKT = L // 128        # 16 k tiles per sequence
NCT = C // 128        # 8 contraction tiles for the projections
NQC = L // 512        # 4 q chunks (groups) per batch
NG = B * NQC          # 16 groups
NF = NG * NKT         # 256 flat steps


def build_kernel():
    nc = bacc.Bacc("TRN2", target_bir_lowering=False, debug=False,
                   num_devices=NCORES)

    xT = nc.dram_tensor("xT", [B, C, L], BF16, kind="ExternalInput")
    # wqkv[j] = [128 c, 1024 (ci,f)]; j in (0=q both heads, 1=k, 2=v)
    wqkv = nc.dram_tensor("wqkv", [3, 128, C], BF16, kind="ExternalInput")
    bqv_d = nc.dram_tensor("bqv", [128, 2], F32, kind="ExternalInput")
    # wo2: [128 c(2 heads), 1024 o]
    wo2 = nc.dram_tensor("wo2", [128, C], BF16, kind="ExternalInput")
    identb_d = nc.dram_tensor("identb_d", [128, 128], BF16, kind="ExternalInput")
    out = nc.dram_tensor("out", [B * L, C], BF16, kind="ExternalOutput")

    with tile.TileContext(nc) as tc:
        kernel_body(nc, tc, xT, wqkv, bqv_d, wo2, identb_d, out)
    nc.compile()
    return nc


def kernel_body(nc, tc, xT, wqkv, bqv_d, wo2, identb_d, out):
    from contextlib import ExitStack
    ctx = ExitStack()
    with ctx:
        consts = ctx.enter_context(tc.tile_pool(name="consts", bufs=1))
        xpool = ctx.enter_context(tc.tile_pool(name="xpool", bufs=16))
        qkvpool = ctx.enter_context(tc.tile_pool(name="qkvpool", bufs=2))
        vppool = ctx.enter_context(tc.tile_pool(name="vppool", bufs=32))
        epool = ctx.enter_context(tc.tile_pool(name="epool", bufs=6))
        zpool = ctx.enter_context(tc.tile_pool(name="zpool", bufs=2))
        cpool = ctx.enter_context(tc.tile_pool(name="cpool", bufs=2))
        opool = ctx.enter_context(tc.tile_pool(name="opool", bufs=4))
        # PSUM: s 2x[128,1024]=4 banks, X 2x[128,512]=2, Y 1, G 1 -> 8 total
        spsum = ctx.enter_context(tc.tile_pool(name="spsum", bufs=2,
                                               space="PSUM"))
        xpsum = ctx.enter_context(tc.tile_pool(name="xpsum", bufs=2,
                                               space="PSUM"))
        ypsum = ctx.enter_context(tc.tile_pool(name="ypsum", bufs=1,
                                               space="PSUM"))
        gpsum = ctx.enter_context(tc.tile_pool(name="gpsum", bufs=1,
                                               space="PSUM"))

        # ---- constants ----
        wj_tiles = []
        for j in range(3):
            t = consts.tile([128, C], BF16, tag=f"wj{j}", name=f"wj{j}")
            nc.sync.dma_start(out=t, in_=wqkv[j])
            wj_tiles.append(t)
        w_tiles = [[wj_tiles[j][:, bass.ts(ci, 128)] for j in range(3)]
                   for ci in range(NCT)]
        bqv_t = consts.tile([128, 2], F32, tag="bqv_t")
        nc.sync.dma_start(out=bqv_t, in_=bqv_d[:])
        bq_t = bqv_t[:, 0:1]
        bv_t = bqv_t[:, 1:2]
        identb = consts.tile([128, 128], BF16, tag="identb")
        nc.sync.dma_start(out=identb, in_=identb_d[:])
        ones_t = consts.tile([128, 1], BF16, tag="ones_t")
        nc.gpsimd.memset(ones_t, 1.0)

        # ---- per-batch state (filled by emit helpers) ----
        qkvT = [None] * B       # [q2, k2, v2] tiles per batch
        vps = [None] * B        # 16 vp tiles per batch
        ctxT = [None] * B       # ctxT2 [128, L] per batch
        xts = [None] * B        # x tiles: dict pair -> list of 8

        def emit_x_loads(b, pair):
            ls = bass.ts(pair, 2 * LCHUNK)
            tiles = []
            for ci in range(NCT):
                xt = xpool.tile([128, 2 * LCHUNK], BF16, tag="xt", name="xt")
                nc.sync.dma_start(out=xt, in_=xT[b, bass.ts(ci, 128), ls])
                tiles.append(xt)
            if xts[b] is None:
                xts[b] = {}
            xts[b][pair] = tiles

        def new_qkvT(b):
            q2 = qkvpool.tile([128, L], BF16, tag="q2", name="q2")
            k2 = qkvpool.tile([128, L], BF16, tag="k2", name="k2")
            v2 = qkvpool.tile([128, L], BF16, tag="v2", name="v2")
            qkvT[b] = [q2, k2, v2]

        def emit_qkv_block(b, lc, j):
            ls = bass.ts(lc, LCHUNK)
            xs = bass.ts(lc % 2, LCHUNK)
            xt8 = xts[b][lc // 2]
            p = gpsum.tile([128, LCHUNK], F32, tag="gpb", name="p")
            for ci in range(NCT):
                nc.tensor.matmul(p, w_tiles[ci][j], xt8[ci][:, xs],
                                 start=(ci == 0), stop=(ci == NCT - 1))
            q2, k2, v2 = qkvT[b]
            if j == 0:
                nc.vector.tensor_scalar_add(q2[:, ls], p, bq_t)
            elif j == 1:
                # k bias dropped: softmax-invariant per query
                nc.vector.tensor_copy(k2[:, ls], p)
            else:
                nc.vector.tensor_scalar_add(v2[:, ls], p, bv_t)

        def emit_transposes(b, kts):
            # v2 [128ch, L] -> vp[kt] [128 tok, 128 ch] via PE transpose
            v2 = qkvT[b][2]
            if vps[b] is None:
                vps[b] = [None] * NKT
            for kt in kts:
                tp = gpsum.tile([128, 128], BF16, tag="gpb", name="tp")
                nc.tensor.transpose(tp, v2[:, bass.ts(kt, 128)], identb[:])
                vp = vppool.tile([128, 128], BF16, tag="vp", name="vp")
                nc.vector.tensor_copy(vp, tp)
                vps[b][kt] = vp

        # ---- attention phases ----
        s_tiles = {}   # f -> psum tile (alive until exp)
        e_tiles = {}   # f -> sbuf e tile (alive until attnV)
        X_cur = [None]  # current group's ctx psum tile
        Y_cur = [None]

        def emit_scores(f):
            g, kt = divmod(f, NKT)
            b, qc = divmod(g, NQC)
            q2, k2, _ = qkvT[b]
            qs = bass.ts(qc, 512)
            s = spsum.tile([128, 1024], F32, tag="s", name="s")
            for h in range(2):
                nc.tensor.matmul(
                    s[:, bass.ts(h, 512)],
                    k2[64 * h:64 * h + 64, bass.ts(kt, 128)],
                    q2[64 * h:64 * h + 64, qs],
                    start=True, stop=True, tile_position=(64 * h, 0))
            e = epool.tile([128, 1024], BF16, tag="e", name="e")
            nc.scalar.activation(e, s, EXP, scale=0.125)
            e_tiles[f] = e

        def emit_attnv(f):
            g, kt = divmod(f, NKT)
            b, qc = divmod(g, NQC)
            if kt == 0:
                X_cur[0] = xpsum.tile([128, 512], F32, tag="X", name="X")
                Y_cur[0] = ypsum.tile([128, 512], F32, tag="Y", name="Y")
            X, Y = X_cur[0], Y_cur[0]
            e = e_tiles.pop(f)
            vp = vps[b][kt]
            st, sp = kt == 0, kt == NKT - 1
            for j in range(4):
                h = j // 2
                nc.tensor.matmul(X[32 * j:32 * (j + 1), :],
                                 vp[:, 32 * j:32 * (j + 1)],
                                 e[:, bass.ts(h, 512)],
                                 start=st, stop=sp, tile_position=(0, 32 * j))
            par = kt % 2
            sth, sph = kt < 2, kt >= NKT - 2
            nc.tensor.matmul(Y[32 * par:32 * par + 1, :], ones_t[:, :],
                             e[:, 0:512], start=sth, stop=sph,
                             tile_position=(0, 32 * par))
            nc.tensor.matmul(Y[64 + 32 * par:64 + 32 * par + 1, :],
                             ones_t[:, :], e[:, 512:1024],
                             start=sth, stop=sph,
                             tile_position=(0, 64 + 32 * par))

        def emit_zchain(g):
            b, qc = divmod(g, NQC)
            X, Y = X_cur[0], Y_cur[0]
            qs = bass.ts(qc, 512)
            yc = zpool.tile([97, 512], F32, tag="yc", name="yc")
            nc.vector.tensor_copy(yc, Y[0:97, :])
            c1 = zpool.tile([1, 512], F32, tag="c1", name="c1")
            c2 = zpool.tile([1, 512], F32, tag="c2", name="c2")
            c3 = zpool.tile([1, 512], F32, tag="c3", name="c3")
            nc.sync.dma_start(out=c1, in_=yc[32:33, :])
            nc.sync.dma_start(out=c2, in_=yc[64:65, :])
            nc.sync.dma_start(out=c3, in_=yc[96:97, :])
            zA = zpool.tile([1, 512], F32, tag="zA", name="zA")
            zB = zpool.tile([1, 512], F32, tag="zB", name="zB")
            nc.vector.tensor_add(zA, yc[0:1, :], c1)
            nc.vector.tensor_add(zB, c2, c3)
            zs2 = zpool.tile([128, 512], F32, tag="zs2", name="zs2")
            zsB = zpool.tile([64, 512], F32, tag="zsB", name="zsB")
            nc.gpsimd.partition_broadcast(zs2[0:64, :], zA[0:1, :])
            nc.gpsimd.partition_broadcast(zsB[0:64, :], zB[0:1, :])
            nc.sync.dma_start(out=zs2[64:128, :], in_=zsB[0:64, :])
            rz2 = zpool.tile([128, 512], F32, tag="rz2", name="rz2")
            nc.vector.reciprocal_approx_fast(out=rz2, in_=zs2)
            nc.vector.tensor_mul(ctxT[b][:, qs], X, rz2)

        def emit_outproj_tile(b, t):
            rows = bass.ds(b * L + t * 128, 128)
            ot = opool.tile([128, C], BF16, tag="ot", name="ot")
            for oc in range(C // 512):
                os_ = bass.ts(oc, 512)
                o = gpsum.tile([128, 512], F32, tag="gpb", name="o")
                nc.tensor.matmul(o, ctxT[b][:, bass.ts(t, 128)],
                                 wo_t[:, os_], start=True, stop=True)
                nc.vector.tensor_copy(ot[:, os_], o)
            nc.sync.dma_start(out=out[rows, :], in_=ot)

        # ---- filler schedule: flat_step -> list of closures ----
        filler = {}

        def sched(f, fn):
            filler.setdefault(f, []).append(fn)

        for b in range(B - 1):
            base = 64 * b
            sched(base + 1, (lambda b=b: emit_x_loads(b + 1, 0)))
            sched(base + 14, (lambda b=b: emit_x_loads(b + 1, 1)))
            sched(base + 2, (lambda b=b: new_qkvT(b + 1)))
            k = 0
            for lc in range(NLC):
                for j in range(3):
                    sched(base + 4 + 4 * k,
                          (lambda b=b, lc=lc, j=j: emit_qkv_block(b + 1, lc, j)))
                    k += 1
            for i in range(4):
                sched(base + 50 + 3 * i,
                      (lambda b=b, i=i: emit_transposes(b + 1,
                                                        range(4 * i, 4 * i + 4))))
        for g in range(NG):
            b, qc = divmod(g, NQC)
            for i in range(4):
                f = 16 * g + 18 + 4 * i
                t = 4 * qc + i
                if f < NF:
                    sched(f, (lambda b=b, t=t: emit_outproj_tile(b, t)))
                # else: epilogue handles it

        # ---- prologue: batch 0 qkv + transposes, first 2 score tiles ----
        emit_x_loads(0, 0)
        emit_x_loads(0, 1)
        wo_t = consts.tile([128, C], BF16, tag="wo_t")
        nc.sync.dma_start(out=wo_t, in_=wo2[:])
        new_qkvT(0)
        for lc in range(NLC):
            for j in range(3):
                emit_qkv_block(0, lc, j)
        emit_transposes(0, range(NKT))
        ctxT[0] = cpool.tile([128, L], BF16, tag="ctxT2", name="ctxT2")
        emit_scores(0)
        emit_scores(1)

        # ---- flat stream ----
        for f in range(NF):
            g, kt = divmod(f, NKT)
            b = g // NQC
            if kt == 0 and g % NQC == 0 and b > 0 and ctxT[b] is None:
                ctxT[b] = cpool.tile([128, L], BF16, tag="ctxT2",
                                     name="ctxT2")
            emit_attnv(f)
            if f + 2 < NF:
                # allocate next batch's ctxT2 before its first normalize
                emit_scores(f + 2)
            if kt == NKT - 1:
                emit_zchain(g)
            for fn in filler.get(f, []):
                fn()

        # ---- epilogue: remaining outproj tiles of the last batch ----
        for i in range(4):
            emit_outproj_tile(B - 1, 12 + i)


_NC_CACHE = None


def get_nc():
    global _NC_CACHE
    if _NC_CACHE is None:
        _NC_CACHE = build_kernel()
    return _NC_CACHE


def prepare_in_maps(x, W_qkv, b_qkv, W_out, b_out):
    x = np.asarray(x, np.float32)
    W_qkv = np.asarray(W_qkv, np.float32)
    b_qkv = np.asarray(b_qkv, np.float32)
    W_out = np.asarray(W_out, np.float32)

    xT = np.ascontiguousarray(x.transpose(0, 2, 1)).astype(ml_dtypes.bfloat16)

    in_maps = []
    for core in range(NCORES):
        h0 = HPC * core
        # per-head channel rows in W_qkv: q = h*192..+64, k = +64, v = +128
        qrows = [np.arange(h * 192, h * 192 + 64) for h in (h0, h0 + 1)]
        krows = [q + 64 for q in qrows]
        vrows = [q + 128 for q in qrows]
        fq = np.concatenate(qrows)
        fk = np.concatenate(krows)
        fv = np.concatenate(vrows)
        # wqkv strips: [j, 128 c, 1024 (ci,f)]
        wt = np.empty((3, 128, C), ml_dtypes.bfloat16)
        for j, rows in enumerate((fq, fk, fv)):
            wT = np.ascontiguousarray(W_qkv[rows].T)  # [1024 c, 128 f]
            # [ci, 128 c, 128 f] -> [128 c, ci*128 + f]
            wt[j] = wT.reshape(NCT, 128, 128).transpose(1, 0, 2).reshape(
                128, C).astype(ml_dtypes.bfloat16)
        # wo2 = [128 c, 1024 o]: rows 0:64 h0 ctx channels, 64:128 h1
        wo2 = np.concatenate([
            np.ascontiguousarray(W_out[:, (h0 + h) * HD:(h0 + h + 1) * HD].T)
            for h in range(HPC)
        ], axis=0)
        in_maps.append({
            "xT": xT,
            "wqkv": wt,
            "bqv": np.ascontiguousarray(
                np.stack([b_qkv[fq], b_qkv[fv]], axis=1), np.float32),
            "wo2": np.ascontiguousarray(wo2).astype(ml_dtypes.bfloat16),
            "identb_d": np.eye(128, dtype=ml_dtypes.bfloat16),
        })
    return in_maps


def kernel(x, W_qkv, b_qkv, W_out, b_out):
    in_maps = prepare_in_maps(x, W_qkv, b_qkv, W_out, b_out)
    res = run_bass_kernel_spmd(get_nc(), in_maps, core_ids=list(range(NCORES)))
    acc = np.zeros((B * L, C), np.float32)
    for core_out in res.results:
        acc += core_out["out"].astype(np.float32)
    acc += np.asarray(b_out, np.float32)[None, :]
    return acc.reshape(B, L, C).astype(np.float32)


if __name__ == "__main__":
    rng = np.random.default_rng(0)
    ins = {
        "x": rng.standard_normal((B, L, C)).astype(np.float32),
        "W_qkv": rng.uniform(-1 / 32, 1 / 32, (3 * C, C)).astype(np.float32),
        "b_qkv": rng.uniform(-1 / 32, 1 / 32, (3 * C,)).astype(np.float32),
        "W_out": rng.uniform(-1 / 32, 1 / 32, (C, C)).astype(np.float32),
        "b_out": rng.uniform(-1 / 32, 1 / 32, (C,)).astype(np.float32),
    }
    o = kernel(**ins)
    print(o.shape, o.dtype)


# revision 3
# speedup vs baseline: 1.0119x; 1.0119x over previous
"""Multi-head attention (B=4, L=2048, C=1024, H=16, HD=64) on 8 NeuronCores.

Sharding: tensor-parallel over heads - 2 heads per core. Each core computes
its heads' QKV projection, attention, and a partial output projection over
its 128 ctx channels; the host sums the 8 partial outputs (b_out added on
host). All HW-side matmul operands are bf16; PSUM accumulation stays fp32.

Single-tiling-mode design (from HW trace analysis of two prior versions):
the PE pays a ~200ns pipeline drain whenever the array tiling mode changes
(row/col split config), and an earlier mixed-mode version lost ~150us to
per-step switches plus HAM cold-clock oscillation (17 throttle events) from
dependency-stall idle windows. So EVERY matmul here runs in the SAME
128x32 column-tiled config (4 independent 32-wide tiles, measured 2.38x+
concurrency on HW); only the 16-per-batch v-transposes use transpose mode,
batched 4-per-slice so their mode switches amortize (~6us total).

  - scores: k strips are zero-padded to the 128 frame on the STATIONARY
    side (kp0 rows 64:128 = 0, kp1 rows 0:64 = 0) so the rhs is the natural
    stacked-heads q2 [128, 512]. 8 strip-matmuls (2 heads x 4 ktok
    sub-strips) over 4 tile positions = 2 concurrent rounds per ktile.
  - attnV: 4 v strips [128 ktok, 32 ch] (both heads, one psum bank X) plus
    2 M=1 "ones" strips at FIXED positions (0,0)/(0,64) accumulating the
    softmax rowsums into Y[0] / Y[64] across all 16 ktiles.
  - Z path (all base-partition-0 APs - HW partition_broadcast breaks
    otherwise, measured): DVE copy Y->SBUF, one DMA gather of row 64,
    gpsimd broadcasts, DMA replicate, reciprocal_approx_fast [128,512],
    one full-width normalize mul. ACT does exp ONLY (the pacing floor:
    256 x [128,1024] exps at ~1.01us).

Pipeline: flat stream of 256 (group, ktile) steps in PAIRS (scores run 2-3
steps ahead of attnV through a 2-deep [128,1024] psum ring); qkv / v
transposes / outproj are filler slices scheduled so the in-order PE queue
always has ready work. PSUM: s 2x[128,1024] + X 2x[128,512] + Y + G = 8.
"""

import numpy as np
import ml_dtypes

import concourse.bass as bass
import concourse.mybir as mybir
import concourse.tile as tile
from concourse import bacc
from concourse.bass_utils import run_bass_kernel_spmd

B, L, C, H, HD = 4, 2048, 1024, 16, 64
NCORES = 8
HPC = H // NCORES  # heads per core = 2
F32 = mybir.dt.float32
BF16 = mybir.dt.bfloat16
EXP = mybir.ActivationFunctionType.Exp

LCHUNK = 512          # token chunk for qkv projection drains
NLC = L // LCHUNK     # 4
NKT = L // 128        # 16 k tiles per sequence
NCT = C // 128        # 8 contraction tiles for the projections
NQC = L // 512        # 4 q chunks (groups) per batch
NG = B * NQC          # 16 groups
NF = NG * NKT         # 256 flat steps


def build_kernel():
    nc = bacc.Bacc("TRN2", target_bir_lowering=False, debug=False,
                   num_devices=NCORES)

    xT = nc.dram_tensor("xT", [B, C, L], BF16, kind="ExternalInput")
    # wqkv[j] = [128 c, 1024 (ci,f)]; j in (0=q both heads, 1=k, 2=v)
    wqkv = nc.dram_tensor("wqkv", [3, 128, C], BF16, kind="ExternalInput")
    bqv_d = nc.dram_tensor("bqv", [128, 2], F32, kind="ExternalInput")
    # wo2: [128 c(2 heads), 1024 o]
    wo2 = nc.dram_tensor("wo2", [128, C], BF16, kind="ExternalInput")
    identb_d = nc.dram_tensor("identb_d", [128, 128], BF16, kind="ExternalInput")
    out = nc.dram_tensor("out", [B * L, C], BF16, kind="ExternalOutput")

    with tile.TileContext(nc) as tc:
        kernel_body(nc, tc, xT, wqkv, bqv_d, wo2, identb_d, out)
    nc.compile()
    return nc


def kernel_body(nc, tc, xT, wqkv, bqv_d, wo2, identb_d, out):
    from contextlib import ExitStack
    ctx = ExitStack()
    with ctx:
        consts = ctx.enter_context(tc.tile_pool(name="consts", bufs=1))
        xpool = ctx.enter_context(tc.tile_pool(name="xpool", bufs=16))
        qkvpool = ctx.enter_context(tc.tile_pool(name="qkvpool", bufs=2))
        vppool = ctx.enter_context(tc.tile_pool(name="vppool", bufs=32))
        epool = ctx.enter_context(tc.tile_pool(name="epool", bufs=6))
        zpool = ctx.enter_context(tc.tile_pool(name="zpool", bufs=2))
        cpool = ctx.enter_context(tc.tile_pool(name="cpool", bufs=2))
        opool = ctx.enter_context(tc.tile_pool(name="opool", bufs=4))
        # PSUM: s 2x[128,1024]=4 banks, X 2x[128,512]=2, Y 1, G 1 -> 8 total
        spsum = ctx.enter_context(tc.tile_pool(name="spsum", bufs=2,
                                               space="PSUM"))
        xpsum = ctx.enter_context(tc.tile_pool(name="xpsum", bufs=2,
                                               space="PSUM"))
        ypsum = ctx.enter_context(tc.tile_pool(name="ypsum", bufs=1,
                                               space="PSUM"))
        gpsum = ctx.enter_context(tc.tile_pool(name="gpsum", bufs=1,
                                               space="PSUM"))

        # ---- constants ----
        wj_tiles = []
        for j in range(3):
            t = consts.tile([128, C], BF16, tag=f"wj{j}", name=f"wj{j}")
            nc.sync.dma_start(out=t, in_=wqkv[j])
            wj_tiles.append(t)
        w_tiles = [[wj_tiles[j][:, bass.ts(ci, 128)] for j in range(3)]
                   for ci in range(NCT)]
        bqv_t = consts.tile([128, 2], F32, tag="bqv_t")
        nc.sync.dma_start(out=bqv_t, in_=bqv_d[:])
        bq_t = bqv_t[:, 0:1]
        bv_t = bqv_t[:, 1:2]
        identb = consts.tile([128, 128], BF16, tag="identb")
        nc.sync.dma_start(out=identb, in_=identb_d[:])
        ones_t = consts.tile([128, 1], BF16, tag="ones_t")
        nc.gpsimd.memset(ones_t, 1.0)

        # ---- per-batch state ----
        qkvT = [None] * B       # [q2, kp0, kp1, v2] per batch
        vps = [None] * B        # 16 vp tiles per batch
        ctxT = [None] * B       # ctxT2 [128, L] per batch
        xts = [None] * B        # pair -> list of 8 x tiles

        def emit_x_loads(b, pair):
            ls = bass.ts(pair, 2 * LCHUNK)
            tiles = []
            for ci in range(NCT):
                xt = xpool.tile([128, 2 * LCHUNK], BF16, tag="xt", name="xt")
                nc.sync.dma_start(out=xt, in_=xT[b, bass.ts(ci, 128), ls])
                tiles.append(xt)
            if xts[b] is None:
                xts[b] = {}
            xts[b][pair] = tiles

        def new_qkvT(b):
            q2 = qkvpool.tile([128, L], BF16, tag="q2", name="q2")
            kp0 = qkvpool.tile([128, L], BF16, tag="kp0", name="kp0")
            kp1 = qkvpool.tile([128, L], BF16, tag="kp1", name="kp1")
            v2 = qkvpool.tile([128, L], BF16, tag="v2", name="v2")
            # zero the dead half of each padded k frame once per instance
            nc.gpsimd.memset(kp0[64:128, :], 0.0)
            nc.gpsimd.memset(kp1[0:64, :], 0.0)
            qkvT[b] = [q2, kp0, kp1, v2]

        def emit_qkv_block(b, lc, j):
            ls = bass.ts(lc, LCHUNK)
            xs = bass.ts(lc % 2, LCHUNK)
            xt8 = xts[b][lc // 2]
            p = gpsum.tile([128, LCHUNK], F32, tag="gpb", name="p")
            # 128x32 col-tiled: 4 output-channel quarters x 8 ci tiles
            for ci in range(NCT):
                for qd in range(4):
                    nc.tensor.matmul(
                        p[32 * qd:32 * (qd + 1), :],
                        w_tiles[ci][j][:, bass.ds(32 * qd, 32)],
                        xt8[ci][:, xs],
                        start=(ci == 0), stop=(ci == NCT - 1),
                        tile_position=(0, 32 * qd))
            q2, kp0, kp1, v2 = qkvT[b]
            if j == 0:
                nc.vector.tensor_scalar_add(q2[:, ls], p, bq_t)
            elif j == 1:
                # k bias dropped (softmax-invariant); split into the two
                # zero-padded stationary frames
                nc.vector.tensor_copy(kp0[0:64, ls], p[0:64, :])
                nc.vector.tensor_copy(kp1[64:128, ls], p[64:128, :])
            else:
                nc.vector.tensor_scalar_add(v2[:, ls], p, bv_t)

        def emit_transposes(b, kts):
            v2 = qkvT[b][3]
            if vps[b] is None:
                vps[b] = [None] * NKT
            for kt in kts:
                tp = gpsum.tile([128, 128], BF16, tag="gpb", name="tp")
                nc.tensor.transpose(tp, v2[:, bass.ts(kt, 128)], identb[:])
                vp = vppool.tile([128, 128], BF16, tag="vp", name="vp")
                nc.vector.tensor_copy(vp, tp)
                vps[b][kt] = vp

        # ---- attention ----
        e_tiles = {}
        X_cur = [None]
        Y_cur = [None]

        def emit_scores(f):
            g, kt = divmod(f, NKT)
            b, qc = divmod(g, NQC)
            q2 = qkvT[b][0]
            kps = (qkvT[b][1], qkvT[b][2])
            qs = bass.ts(qc, 512)
            s = spsum.tile([128, 1024], F32, tag="s", name="s")
            for h in range(2):
                kp = kps[h]
                for j in range(4):
                    nc.tensor.matmul(
                        s[32 * j:32 * (j + 1), bass.ts(h, 512)],
                        kp[:, bass.ds(kt * 128 + 32 * j, 32)],
                        q2[:, qs],
                        start=True, stop=True, tile_position=(0, 32 * j))
            e = epool.tile([128, 1024], BF16, tag="e", name="e")
            nc.scalar.activation(e, s, EXP, scale=0.125)
            e_tiles[f] = e

        def emit_attnv(f):
            g, kt = divmod(f, NKT)
            b, qc = divmod(g, NQC)
            if kt == 0:
                X_cur[0] = xpsum.tile([128, 512], F32, tag="X", name="X")
                Y_cur[0] = ypsum.tile([128, 512], F32, tag="Y", name="Y")
            X, Y = X_cur[0], Y_cur[0]
            e = e_tiles.pop(f)
            vp = vps[b][kt]
            st, sp = kt == 0, kt == NKT - 1
            for j in range(4):
                h = j // 2
                nc.tensor.matmul(X[32 * j:32 * (j + 1), :],
                                 vp[:, 32 * j:32 * (j + 1)],
                                 e[:, bass.ts(h, 512)],
                                 start=st, stop=sp, tile_position=(0, 32 * j))
            # rowsums: fixed positions -> Y[0] (h0), Y[64] (h1)
            nc.tensor.matmul(Y[0:1, :], ones_t[:, :], e[:, 0:512],
                             start=st, stop=sp, tile_position=(0, 0))
            nc.tensor.matmul(Y[64:65, :], ones_t[:, :], e[:, 512:1024],
                             start=st, stop=sp, tile_position=(0, 64))

        def emit_zchain(g):
            b, qc = divmod(g, NQC)
            X, Y = X_cur[0], Y_cur[0]
            qs = bass.ts(qc, 512)
            yc = zpool.tile([65, 512], F32, tag="yc", name="yc")
            nc.vector.tensor_copy(yc, Y[0:65, :])
            c2 = zpool.tile([1, 512], F32, tag="c2", name="c2")
            nc.sync.dma_start(out=c2, in_=yc[64:65, :])
            zs2 = zpool.tile([128, 512], F32, tag="zs2", name="zs2")
            zsB = zpool.tile([64, 512], F32, tag="zsB", name="zsB")
            nc.gpsimd.partition_broadcast(zs2[0:64, :], yc[0:1, :])
            nc.gpsimd.partition_broadcast(zsB[0:64, :], c2[0:1, :])
            nc.sync.dma_start(out=zs2[64:128, :], in_=zsB[0:64, :])
            rz2 = zpool.tile([128, 512], F32, tag="rz2", name="rz2")
            nc.vector.reciprocal_approx_fast(out=rz2, in_=zs2)
            nc.vector.tensor_mul(ctxT[b][:, qs], X, rz2)

        def emit_outproj_tile(b, t):
            rows = bass.ds(b * L + t * 128, 128)
            ot = opool.tile([128, C], BF16, tag="ot", name="ot")
            for oc in range(C // 512):
                os_ = bass.ts(oc, 512)
                o = gpsum.tile([128, 512], F32, tag="gpb", name="o")
                for qd in range(4):
                    nc.tensor.matmul(
                        o[32 * qd:32 * (qd + 1), :],
                        ctxT[b][:, bass.ds(t * 128 + 32 * qd, 32)],
                        wo_t[:, os_], start=True, stop=True,
                        tile_position=(0, 32 * qd))
                nc.vector.tensor_copy(ot[:, os_], o)
            nc.sync.dma_start(out=out[rows, :], in_=ot)

        # ---- filler schedule ----
        filler = {}

        def sched(f, fn):
            filler.setdefault(f, []).append(fn)

        for b in range(B - 1):
            base = 64 * b
            sched(base + 1, (lambda b=b: emit_x_loads(b + 1, 0)))
            sched(base + 14, (lambda b=b: emit_x_loads(b + 1, 1)))
            sched(base + 2, (lambda b=b: new_qkvT(b + 1)))
            k = 0
            for lc in range(NLC):
                for j in range(3):
                    sched(base + 4 + 4 * k,
                          (lambda b=b, lc=lc, j=j: emit_qkv_block(b + 1, lc, j)))
                    k += 1
            for i in range(4):
                sched(base + 50 + 3 * i,
                      (lambda b=b, i=i: emit_transposes(b + 1,
                                                        range(4 * i, 4 * i + 4))))
        for g in range(NG):
            b, qc = divmod(g, NQC)
            for i in range(4):
                f = 16 * g + 22 + 2 * i
                t = 4 * qc + i
                if f < NF:
                    sched(f, (lambda b=b, t=t: emit_outproj_tile(b, t)))

        # ---- prologue ----
        emit_x_loads(0, 0)
        emit_x_loads(0, 1)
        wo_t = consts.tile([128, C], BF16, tag="wo_t")
        nc.sync.dma_start(out=wo_t, in_=wo2[:])
        new_qkvT(0)
        for lc in range(NLC):
            for j in range(3):
                emit_qkv_block(0, lc, j)
        emit_transposes(0, range(NKT))
        ctxT[0] = cpool.tile([128, L], BF16, tag="ctxT2", name="ctxT2")
        emit_scores(0)
        emit_scores(1)

        # ---- flat stream, pair-batched ----
        for f2 in range(0, NF, 2):
            for f in (f2, f2 + 1):
                g, kt = divmod(f, NKT)
                b = g // NQC
                if kt == 0 and g % NQC == 0 and b > 0 and ctxT[b] is None:
                    ctxT[b] = cpool.tile([128, L], BF16, tag="ctxT2",
                                         name="ctxT2")
                emit_attnv(f)
                if kt == NKT - 1:
                    emit_zchain(g)
            for f in (f2 + 2, f2 + 3):
                if f < NF:
                    emit_scores(f)
            for f in (f2, f2 + 1):
                for fn in filler.get(f, []):
                    fn()

        # ---- epilogue ----
        for i in range(4):
            emit_outproj_tile(B - 1, 12 + i)


_NC_CACHE = None


def get_nc():
    global _NC_CACHE
    if _NC_CACHE is None:
        _NC_CACHE = build_kernel()
    return _NC_CACHE


def prepare_in_maps(x, W_qkv, b_qkv, W_out, b_out):
    x = np.asarray(x, np.float32)
    W_qkv = np.asarray(W_qkv, np.float32)
    b_qkv = np.asarray(b_qkv, np.float32)
    W_out = np.asarray(W_out, np.float32)

    xT = np.ascontiguousarray(x.transpose(0, 2, 1)).astype(ml_dtypes.bfloat16)

    in_maps = []
    for core in range(NCORES):
        h0 = HPC * core
        # per-head channel rows in W_qkv: q = h*192..+64, k = +64, v = +128
        qrows = [np.arange(h * 192, h * 192 + 64) for h in (h0, h0 + 1)]
        krows = [q + 64 for q in qrows]
        vrows = [q + 128 for q in qrows]
        fq = np.concatenate(qrows)
        fk = np.concatenate(krows)
        fv = np.concatenate(vrows)
        # wqkv strips: [j, 128 c, 1024 (ci,f)]
        wt = np.empty((3, 128, C), ml_dtypes.bfloat16)
        for j, rows in enumerate((fq, fk, fv)):
            wT = np.ascontiguousarray(W_qkv[rows].T)  # [1024 c, 128 f]
            # [ci, 128 c, 128 f] -> [128 c, ci*128 + f]
            wt[j] = wT.reshape(NCT, 128, 128).transpose(1, 0, 2).reshape(
                128, C).astype(ml_dtypes.bfloat16)
        # wo2 = [128 c, 1024 o]: rows 0:64 h0 ctx channels, 64:128 h1
        wo2 = np.concatenate([
            np.ascontiguousarray(W_out[:, (h0 + h) * HD:(h0 + h + 1) * HD].T)
            for h in range(HPC)
        ], axis=0)
        in_maps.append({
            "xT": xT,
            "wqkv": wt,
            "bqv": np.ascontiguousarray(
                np.stack([b_qkv[fq], b_qkv[fv]], axis=1), np.float32),
            "wo2": np.ascontiguousarray(wo2).astype(ml_dtypes.bfloat16),
            "identb_d": np.eye(128, dtype=ml_dtypes.bfloat16),
        })
    return in_maps


def kernel(x, W_qkv, b_qkv, W_out, b_out):
    in_maps = prepare_in_maps(x, W_qkv, b_qkv, W_out, b_out)
    res = run_bass_kernel_spmd(get_nc(), in_maps, core_ids=list(range(NCORES)))
    acc = np.zeros((B * L, C), np.float32)
    for core_out in res.results:
        acc += core_out["out"].astype(np.float32)
    acc += np.asarray(b_out, np.float32)[None, :]
    return acc.reshape(B, L, C).astype(np.float32)


if __name__ == "__main__":
    rng = np.random.default_rng(0)
    ins = {
        "x": rng.standard_normal((B, L, C)).astype(np.float32),
        "W_qkv": rng.uniform(-1 / 32, 1 / 32, (3 * C, C)).astype(np.float32),
        "b_qkv": rng.uniform(-1 / 32, 1 / 32, (3 * C,)).astype(np.float32),
        "W_out": rng.uniform(-1 / 32, 1 / 32, (C, C)).astype(np.float32),
        "b_out": rng.uniform(-1 / 32, 1 / 32, (C,)).astype(np.float32),
    }
    o = kernel(**ins)
    print(o.shape, o.dtype)
